# revision 75
# baseline (speedup 1.0000x reference)
"""DeepNCM Trainium2 kernel: prototype scatter-mean update + negative squared
L2 distances, data-parallel over embedding rows across 8 NeuronCores.

Contract: kernel(**inputs) takes the FULL unsharded inputs
(embeddings [65536,512] f32, prototypes [1000,512] f32, counter [1000] f32,
y_true [65536] int64) and returns the FULL output [65536,1000] f32.

Per-core plan (NL = 8192 rows, class axis padded to 1024 = 8 chunks of 128,
fp8 DoubleRow matmuls throughout; ~148us in the CoreSim cost model vs 365us
for the bf16 AllReduce baseline):
  Phase 1: sumsT[c, d] = oh^T @ emb_aug via DoubleRow fp8 matmuls (two
     row-tiles = 256 contraction rows per instruction), one pr-major pass
     over all 8 class chunks (8 PSUM banks), pipelined with one-hot
     generation (DVE+Pool). emb_aug carries a ones column; the per-class
     counts matmuls reuse each chunk's bank as its sums copy drains it.
     e_sq comes from bf16 emb via Act Square+accumulate, running through
     the collective windows.
  ReduceScatter [1024,516] bf16 -> each rank owns 128 classes (sums+counts).
  Per-rank prototype update + PE-transpose to [d, c] BEFORE the AllGather,
  so the fp8 AG payload [513,128] lands in matmul-ready layout (plus a
  -p_sq row); nothing but strided loads remains after the AG.
  Phase 2: out = 2*emb@protosT - e_sq - p_sq: each PSUM half accumulates a
     K=2 DoubleRow matmul seeding -p_sq, then 2 DoubleRow fp8 matmuls
     (embT pairs x protos2); the epilogue copy adds -e_sq (per-partition
     bias/scalar) split Act/DVE; output DMAs rotate across SP/Pool/Act.
"""

import os
import sys
from contextlib import ExitStack

for _p in ("/opt/trn_rl_repo", "/root/.axon_site/_ro/trn_rl_repo"):
    if os.path.isdir(_p):
        if _p not in sys.path:
            sys.path.insert(0, _p)
        break

import numpy as np
import ml_dtypes

import concourse.bass as bass
import concourse.mybir as mybir
import concourse.tile as tile
from concourse.masks import make_identity
from concourse.bass_utils import run_bass_kernel_spmd

N, D, C = 65536, 512, 1000
W = 8                      # cores
NL = N // W                # 8192 rows per core
P = 128
KT = NL // P               # 64 row tiles per core
PR = KT // 2               # 32 row-tile pairs (DoubleRow)
CP = 1024                  # class axis padded to 8 chunks of 128
CH = 128                   # classes per rank / class chunk (incl. padding)
FA = 520                   # emb_aug width: 512 emb + ones col + 7 pad
FU = 516                   # used width in collective buffers
F32 = mybir.dt.float32
BF16 = mybir.dt.bfloat16
F8 = mybir.dt.float8e4
ALU = mybir.AluOpType
ACTF = mybir.ActivationFunctionType
DRM = mybir.MatmulPerfMode.DoubleRow

# Toggled by test.py for profiling runs.
PROFILE = False
TRACE_KWARGS = {}
LAST_RESULT = [None]

_built = [None]


def _split_waits(nc, cap=1):
    """Walrus in this container rejects >1 sync-wait per instruction.
    Move excess waits onto preceding same-engine NOPs (in-order engines,
    so semantics are preserved)."""
    n_new = 0
    for fn in nc.m.functions:
        for bb in fn.blocks:
            new_list = []
            for ins in bb.instructions:
                si = getattr(ins, "sync_info", None)
                if si is not None and si.on_wait and len(si.on_wait) > cap:
                    waits = list(si.on_wait)
                    keep, rest = waits[:cap], waits[cap:]
                    for i in range(0, len(rest), cap):
                        nop = mybir.InstNoOp(
                            name=f"I-waitsplit-{n_new}", ins=[], outs=[]
                        )
                        n_new += 1
                        nop.engine = ins.engine
                        nop.sync_info = mybir.SyncInfo(
                            on_wait=rest[i : i + cap], on_update=[]
                        )
                        new_list.append(nop)
                    si.on_wait = keep
                new_list.append(ins)
            bb.instructions = new_list
    return n_new


def _build():
    nc = bass.Bass()
    ea_ext = nc.declare_dram_parameter("ea", [PR, P, 2 * FA], F8, isOutput=False)
    eb_ext = nc.declare_dram_parameter("eb", [PR, P, 2 * D], BF16, isOutput=False)
    et_ext = nc.declare_dram_parameter("et", [P, 4 * NL], F8, isOutput=False)
    yf_ext = nc.declare_dram_parameter("yf", [P, KT], F32, isOutput=False)
    ctr_ext = nc.declare_dram_parameter("ctr", [CH, 1], F32, isOutput=False)
    p0s_ext = nc.declare_dram_parameter("p0s", [CH, D], F32, isOutput=False)
    out_ext = nc.declare_dram_parameter("out", [NL, C], F32, isOutput=True)

    with tile.TileContext(nc) as tc, ExitStack() as es:
        cpool = es.enter_context(tc.tile_pool(name="const", bufs=1))
        bpool = es.enter_context(tc.tile_pool(name="bigs", bufs=1))
        bigp = es.enter_context(tc.tile_pool(name="bigp", bufs=1))
        ebp = es.enter_context(tc.tile_pool(name="ebp", bufs=20))
        sqp = es.enter_context(tc.tile_pool(name="sqp", bufs=2))
        rp = es.enter_context(tc.tile_pool(name="rp", bufs=1))
        otp = es.enter_context(tc.tile_pool(name="otp", bufs=8))
        dram = es.enter_context(tc.tile_pool(name="dram", bufs=1, space="DRAM"))

        # ---- constants ----
        iota = cpool.tile([P, CP], F32, name="iota")
        nc.gpsimd.iota(
            iota[:], pattern=[[1, CP]], base=0, channel_multiplier=0,
            allow_small_or_imprecise_dtypes=True,
        )
        identb = cpool.tile([P, P], BF16, name="identb")
        make_identity(nc, identb[:])
        ones2b = cpool.tile([2, 2 * P], F8, name="ones2b")
        nc.vector.memset(ones2b[:], 0.0)
        nc.vector.memset(ones2b[0:1, 0:P], 1.0)

        y_sb = cpool.tile([P, KT], F32, name="y")
        nc.sync.dma_start(y_sb[:], yf_ext[:])
        ctr_sb = rp.tile([CH, 1], F32, name="ctr")
        nc.sync.dma_start(ctr_sb[:], ctr_ext[:])
        p0s_sb = cpool.tile([CH, D], F32, name="p0s")
        nc.sync.dma_start(p0s_sb[:], p0s_ext[:])
        e_sq = cpool.tile([P, KT], F32, name="esq")

        # ---- big resident buffers ----
        # ea_full (phase 1) and embT (phase 2) are never live at the same
        # time: share one pool slot (same tag) to free 32KB/partition.
        ea_full = bigp.tile([P, PR * 2 * FA], F8, tag="big", name="ea")
        eav = ea_full.rearrange("p (pr j f) -> p pr j f", pr=PR, j=2)
        oh_full = bpool.tile([P, KT * CP], F8, name="oh")
        ohv = oh_full.rearrange("p (pr j c) -> p pr j c", pr=PR, j=2)
        p2sb = bpool.tile([P, 4 * CP], F8, name="p2sb")
        p2v = p2sb.rearrange("p (dc c) -> p dc c", dc=4)  # c = CP
        psq2b = cpool.tile([2, 2 * CP], F8, name="psq2b")
        nc.vector.memset(psq2b[:], 0.0)
        ss = bpool.tile([P, 8 * FU], BF16, name="ss")
        ssv = ss.rearrange("p (cc f) -> p cc f", cc=8)

        # collective DRAM buffers
        ccin = dram.tile([CP, FU], BF16, name="ccin")
        rsout = dram.tile([CH, FU], BF16, name="rsout")
        agin = dram.tile([513, P], F8, name="agin")
        agout = dram.tile([W * 513, P], F8, name="agout", addr_space="Shared")

        # ---- phase 1: loads + one-hot ----
        eb_tiles = []
        for pr in range(PR):
            nc.sync.dma_start(
                eav[:, pr, :, :], ea_ext[pr]
            )
            for j in (0, 1):
                kt = 2 * pr + j
                dst = ohv[:, pr, j, :]
                # split one-hot generation DVE : Pool roughly 39:25
                eng = nc.vector if (kt % 16) < 10 else nc.gpsimd
                eng.tensor_scalar(dst, iota[:], y_sb[:, kt : kt + 1], None,
                                  ALU.is_equal)

        # counter-only coefficient work hoisted ahead of the ReduceScatter
        rt2 = rp.tile([CH, 1], F32, name="rt2")
        nc.vector.tensor_scalar(rt2[:], ctr_sb[:], 1.0, None, ALU.add)
        nc.vector.reciprocal(rt2[:], rt2[:])
        A2p = rp.tile([CH, 1], F32, name="A2p")
        nc.vector.tensor_tensor(out=A2p[:], in0=ctr_sb[:], in1=rt2[:], op=ALU.mult)
        nc.vector.tensor_scalar(A2p[:], A2p[:], 1.0, None, ALU.subtract)
        nc.vector.tensor_scalar(A2p[:], A2p[:], 2.0, None, ALU.mult)
        nc.vector.tensor_scalar(rt2[:], rt2[:], 2.0, None, ALU.mult)
        twos_c = rp.tile([CH, 1], F32, name="twosc")
        nc.vector.memset(twos_c[:], 2.0)


        # ---- phase 1: segment sums via DoubleRow fp8 ----
        # pr-major main pass over all 8 class chunks (8 PSUM banks),
        # pipelined with one-hot generation; the last 4 row-pairs of each
        # chunk run as staggered per-chunk tails so copies, counts matmuls
        # (bank reuse) and ccin DMAs pipeline instead of all colliding at
        # the end of the pass.
        PRM = PR - 6
        with tc.tile_pool(name="ps_seg", bufs=1, space="PSUM") as psg:
            psAs = {cc: psg.tile([CH, 512], F32, tag=f"psA{cc}",
                                 name=f"psA{cc}") for cc in range(8)}
            for pr in range(PRM):
                for cc in range(8):
                    nc.tensor.matmul(
                        psAs[cc][:], ohv[:, pr, :, cc * CH : (cc + 1) * CH],
                        eav[:, pr, :, 0:512],
                        start=(pr == 0), stop=False, perf_mode=DRM,
                    )
            for cc in range(8):
                for pr in range(PRM, PR):
                    nc.tensor.matmul(
                        psAs[cc][:], ohv[:, pr, :, cc * CH : (cc + 1) * CH],
                        eav[:, pr, :, 0:512],
                        start=False, stop=(pr == PR - 1), perf_mode=DRM,
                    )
                # sums copy on DVE (Act runs the e_sq squares; Pool must stay
                # clear so the ReduceScatter can start early)
                nc.vector.tensor_copy(out=ssv[:, cc, 0:512], in_=psAs[cc][:])
                # counts: reuse chunk cc's bank (same tag) once copied out
                psB = psg.tile([CH, 512], F32, tag=f"psA{cc}", name=f"psB{cc}")
                for pr in range(PR):
                    nc.tensor.matmul(
                        psB[:, 0:4], ohv[:, pr, :, cc * CH : (cc + 1) * CH],
                        eav[:, pr, :, 512:516],
                        start=(pr == 0), stop=(pr == PR - 1), perf_mode=DRM,
                    )
                nc.vector.tensor_copy(out=ssv[:, cc, 512:516], in_=psB[:, 0:4])
                # ccin DMAs ride the Pool queue (SP is busy with eb loads;
                # Pool is idle between one-hot gen and the ReduceScatter)
                nc.gpsimd.dma_start(ccin[cc * CH : (cc + 1) * CH, :],
                                    ssv[:, cc, :])

        # ---- ReduceScatter (sums+counts, bf16) ----
        nc.gpsimd.collective_compute(
            "ReduceScatter", ALU.add,
            replica_groups=[list(range(W))],
            ins=[ccin.opt()], outs=[rsout.opt()],
        )

        # ---- e_sq: eb loads on SP; Square+accumulate on Act, which is
        # otherwise idle and keeps running through the collective windows ----
        for pr in range(PR):
            ebt = ebp.tile([P, 2 * D], BF16, tag="eb", name="eb")
            nc.sync.dma_start(ebt[:], eb_ext[pr])
            eb_tiles.append(ebt)
            for j in (0, 1):
                kt = 2 * pr + j
                scr = sqp.tile([P, D], BF16, tag="scr", name="scr")
                nc.scalar.activation(
                    scr[:], ebt[:, j * D : (j + 1) * D], ACTF.Square,
                    accum_out=e_sq[:, kt : kt + 1],
                )

        # negate e_sq once (used as per-partition bias in phase 2)
        nc.scalar.mul(e_sq[:], e_sq[:], -1.0)

        # ---- embT load into ea_full's slot (overlaps the ReduceScatter) ----
        embT = bigp.tile([P, PR * 2 * FA], F8, tag="big", name="embT")
        etv = embT.rearrange("p (q n) -> p q n", q=4)[:, :, 0:NL]
        for q in range(4):
            nc.sync.dma_start(etv[:, q, :], et_ext[:, q * NL : (q + 1) * NL])

        # ---- per-rank prototype update (128 classes incl. padding) ----
        # B2 = 2*rep*rm*rt ; A2 = 2*(1 + rep*(ctr*rt - 1)); rt2=2rt and
        # A2p=ctr*rt-1 were precomputed before the ReduceScatter.
        shard = rp.tile([CH, FU], BF16, name="shard")
        nc.sync.dma_start(shard[:], rsout[:])
        counts = shard[:, 512:513]
        rm = rp.tile([CH, 1], F32, name="rm")
        nc.vector.tensor_scalar(rm[:], counts, 1.0, None, ALU.max)
        nc.vector.reciprocal(rm[:], rm[:])
        rep = rp.tile([CH, 1], F32, name="rep")
        nc.vector.tensor_scalar(rep[:], counts, 0.0, None, ALU.is_gt)
        B2 = rp.tile([CH, 1], F32, name="B2")
        nc.vector.scalar_tensor_tensor(
            out=B2[:], in0=rm[:], scalar=rt2[:], in1=rep[:],
            op0=ALU.mult, op1=ALU.mult,
        )
        A2 = rp.tile([CH, 1], F32, name="A2")
        nc.vector.scalar_tensor_tensor(
            out=A2[:], in0=A2p[:], scalar=rep[:], in1=twos_c[:],
            op0=ALU.mult, op1=ALU.add,
        )

        tB = rp.tile([CH, D], F32, name="tB")
        nc.vector.tensor_scalar(tB[:], shard[:, 0:512], B2[:], None, ALU.mult)
        p2t_b = rp.tile([CH, D], BF16, name="p2tb")
        nc.vector.scalar_tensor_tensor(
            out=p2t_b[:], in0=p0s_sb[:], scalar=A2[:], in1=tB[:],
            op0=ALU.mult, op1=ALU.add,
        )
        # -p_sq = -0.25 * sum_d protos2^2
        scr2 = rp.tile([CH, D], BF16, name="scr2")
        npsq_b = rp.tile([CH, 1], BF16, name="npsqb")
        nc.vector.scalar_tensor_tensor(
            out=scr2[:], in0=p2t_b[:], scalar=-0.25, in1=p2t_b[:],
            op0=ALU.mult, op1=ALU.mult, accum_out=npsq_b[:],
        )

        # transpose this rank's protos2T to [d, c] BEFORE the AllGather so
        # no transpose work sits on the post-collective critical path
        agst = rp.tile([P, 4 * P], F8, name="agst")
        agsv = agst.rearrange("p (dc c) -> p dc c", dc=4)
        psq_st = rp.tile([1, P], F8, name="psqst")
        with tc.tile_pool(name="ps_tr", bufs=1, space="PSUM") as pst:
            t2 = pst.tile([P, 4 * P], BF16, tag="t2", name="t2")
            t2v = t2.rearrange("p (dc c) -> p dc c", dc=4)
            for dc in range(4):
                nc.tensor.matmul(
                    t2v[:, dc, :], p2t_b[:, dc * P : (dc + 1) * P], identb[:],
                    is_transpose=True, start=(dc == 0), stop=(dc == 3),
                )
            tq2 = pst.tile([1, P], BF16, tag="tq2", name="tq2")
            nc.tensor.matmul(tq2[:], npsq_b[:], identb[:],
                             is_transpose=True, start=True, stop=True)
            nc.vector.tensor_copy(out=agst[:], in_=t2[:])
            nc.vector.tensor_copy(out=psq_st[:], in_=tq2[:])
        # agin rows 0..511 = protos2 chunk [d, c]; row 512 = -p_sq row
        nc.sync.dma_start(
            agin[0:512, :].rearrange("(dc p) c -> p dc c", dc=4),
            agsv[:, :, :],
        )
        nc.sync.dma_start(agin[512:513, :], psq_st[:])

        # ---- AllGather (protos2T + -p_sq, fp8) ----
        nc.gpsimd.collective_compute(
            "AllGather", ALU.bypass,
            replica_groups=[list(range(W))],
            ins=[agin.opt()], outs=[agout.opt()],
        )

        # ---- load gathered protos2 (already [d, c] per rank) + -p_sq row ----
        # psq row first: the p_sq seed matmul opens every accumulation group
        agov = agout.rearrange("(r q) c -> q r c", r=W)
        nc.sync.dma_start(
            psq2b[0:1, 0:CP].rearrange("a (r c) -> a r c", r=W),
            agov[512:513, :, :],
        )
        for dc in range(4):
            eng = nc.scalar if dc < 2 else nc.sync
            eng.dma_start(
                p2sb.rearrange("p (dc r c) -> p dc r c", dc=4, r=W)[:, dc, :, :],
                agov[dc * P : (dc + 1) * P, :, :],
            )

        # ---- phase 2: out = 2*emb@protosT - e_sq - p_sq ----
        with tc.tile_pool(name="ps_cr", bufs=4, space="PSUM") as ps_cr:
            for nt in range(KT):
                ot = otp.tile([P, CP], F32, tag="ot", name="ot")
                for h in range(2):
                    c0 = 512 * h
                    cr = ps_cr.tile([P, 512], F32, tag=f"cr{h}", name=f"cr{h}")
                    nc.tensor.matmul(
                        cr[:],
                        ones2b.rearrange("k (j m) -> k j m", j=2)[:, :, :],
                        psq2b.rearrange("k (j c) -> k j c", j=2)[:, :, c0 : c0 + 512],
                        start=True, stop=False, perf_mode=DRM,
                    )
                    for q in range(2):
                        nc.tensor.matmul(
                            cr[:],
                            etv[:, 2 * q : 2 * q + 2, nt * P : (nt + 1) * P],
                            p2v[:, 2 * q : 2 * q + 2, c0 : c0 + 512],
                            start=False, stop=(q == 1), perf_mode=DRM,
                        )
                    # epilogue: add -e_sq while copying psum -> sbuf
                    # (gpsimd cannot access PSUM, so Act/DVE only)
                    if (2 * nt + h) % 9 in (0, 2, 4, 6):
                        nc.scalar.activation(
                            ot[:, c0 : c0 + 512], cr[:], ACTF.Identity,
                            bias=e_sq[:, nt : nt + 1], scale=1.0,
                        )
                    else:
                        nc.vector.tensor_scalar(
                            ot[:, c0 : c0 + 512], cr[:], e_sq[:, nt : nt + 1],
                            None, ALU.add,
                        )
                # output DMA rotation, finely interleaved: SP ~28, Pool ~28, Act ~8
                r = nt % 9
                if r in (0, 2, 4, 6):
                    eng = nc.sync
                elif r in (1, 3, 5, 7):
                    eng = nc.gpsimd
                else:
                    eng = nc.scalar
                eng.dma_start(out_ext[nt * P : (nt + 1) * P, :], ot[:, 0:C])

    _split_waits(nc)
    return nc


def _prep_inputs(embeddings, prototypes, counter, y_true):
    """Host-side sharding + layout prep (no kernel math beyond dtype casts)."""
    emb = np.ascontiguousarray(np.asarray(embeddings, dtype=np.float32))
    p0 = np.ascontiguousarray(np.asarray(prototypes, dtype=np.float32))
    ctr = np.ascontiguousarray(np.asarray(counter, dtype=np.float32))
    y = np.asarray(y_true)

    f8 = ml_dtypes.float8_e4m3
    bf = ml_dtypes.bfloat16

    p0_pad = np.zeros((CP, D), dtype=np.float32)
    p0_pad[0:C] = p0
    ctr_pad = np.zeros((CP,), dtype=np.float32)
    ctr_pad[0:C] = ctr

    in_maps = []
    for i in range(W):
        sl = slice(i * NL, (i + 1) * NL)
        e_i = emb[sl]                                   # [NL, D] f32
        # emb_aug fp8 pairs, partition-major: [PR, P, 2*FA]
        ea = np.zeros((NL, FA), dtype=f8)
        ea[:, 0:D] = e_i.astype(f8)
        ea[:, D] = 1.0
        ea_t = np.ascontiguousarray(
            ea.reshape(PR, 2, P, FA).transpose(0, 2, 1, 3).reshape(PR, P, 2 * FA)
        )
        # bf16 pairs for e_sq: [PR, P, 2*D]
        eb = e_i.astype(bf)
        eb_t = np.ascontiguousarray(
            eb.reshape(PR, 2, P, D).transpose(0, 2, 1, 3).reshape(PR, P, 2 * D)
        )
        # embT fp8: [P, 4*NL] with et[k, dc*NL + n] = emb[n, 128*dc + k]
        et = np.ascontiguousarray(
            e_i.astype(f8).T.reshape(4, P, NL).transpose(1, 0, 2).reshape(P, 4 * NL)
        )
        # labels, partition-major: yf[p, t] = y[t*128 + p]
        y_loc = y[sl].astype(np.float32)
        yf = np.ascontiguousarray(y_loc.reshape(KT, P).T)
        # per-rank class shard (class axis padded to CP)
        cs = slice(i * CH, (i + 1) * CH)
        in_maps.append(
            {
                "ea": ea_t,
                "eb": eb_t,
                "et": et,
                "yf": yf,
                "ctr": np.ascontiguousarray(ctr_pad[cs]).reshape(CH, 1),
                "p0s": np.ascontiguousarray(p0_pad[cs]),
            }
        )
    return in_maps


def kernel(embeddings, prototypes, counter, y_true):
    if _built[0] is None:
        _built[0] = _build()
    nc = _built[0]

    in_maps = _prep_inputs(embeddings, prototypes, counter, y_true)

    res = run_bass_kernel_spmd(
        nc, in_maps, list(range(W)), trace=PROFILE, **TRACE_KWARGS
    )
    LAST_RESULT[0] = res
    out = np.concatenate([res.results[i]["out"] for i in range(W)], axis=0)
    return out.astype(np.float32, copy=False)


# revision 80
# speedup vs baseline: 1.0117x; 1.0117x over previous
"""DeepNCM Trainium2 kernel: prototype scatter-mean update + negative squared
L2 distances, data-parallel over embedding rows across 8 NeuronCores.

Contract: kernel(**inputs) takes the FULL unsharded inputs
(embeddings [65536,512] f32, prototypes [1000,512] f32, counter [1000] f32,
y_true [65536] int64) and returns the FULL output [65536,1000] f32.

Per-core plan (NL = 8192 rows, class axis padded to 1024 = 8 chunks of 128,
fp8 DoubleRow matmuls throughout; ~148us in the CoreSim cost model vs 365us
for the bf16 AllReduce baseline):
  Phase 1: sumsT[c, d] = oh^T @ emb_aug via DoubleRow fp8 matmuls (two
     row-tiles = 256 contraction rows per instruction), one pr-major pass
     over all 8 class chunks (8 PSUM banks), pipelined with one-hot
     generation (DVE+Pool). emb_aug carries a ones column; the per-class
     counts matmuls reuse each chunk's bank as its sums copy drains it.
     e_sq comes from bf16 emb via Act Square+accumulate, running through
     the collective windows.
  ReduceScatter [1024,516] bf16 -> each rank owns 128 classes (sums+counts).
  Per-rank prototype update + PE-transpose to [d, c] BEFORE the AllGather,
  so the fp8 AG payload [513,128] lands in matmul-ready layout (plus a
  -p_sq row); nothing but strided loads remains after the AG.
  Phase 2: out = 2*emb@protosT - e_sq - p_sq: each PSUM half accumulates a
     K=2 DoubleRow matmul seeding -p_sq, then 2 DoubleRow fp8 matmuls
     (embT pairs x protos2); the epilogue copy adds -e_sq (per-partition
     bias/scalar) split Act/DVE; output DMAs rotate across SP/Pool/Act.
"""

import os
import sys
from contextlib import ExitStack

for _p in ("/opt/trn_rl_repo", "/root/.axon_site/_ro/trn_rl_repo"):
    if os.path.isdir(_p):
        if _p not in sys.path:
            sys.path.insert(0, _p)
        break

import numpy as np
import ml_dtypes

import concourse.bass as bass
import concourse.mybir as mybir
import concourse.tile as tile
from concourse.masks import make_identity
from concourse.bass_utils import run_bass_kernel_spmd

N, D, C = 65536, 512, 1000
W = 8                      # cores
NL = N // W                # 8192 rows per core
P = 128
KT = NL // P               # 64 row tiles per core
PR = KT // 2               # 32 row-tile pairs (DoubleRow)
CP = 1024                  # class axis padded to 8 chunks of 128
CH = 128                   # classes per rank / class chunk (incl. padding)
FA = 520                   # emb_aug width: 512 emb + ones col + 7 pad
FU = 516                   # used width in collective buffers
F32 = mybir.dt.float32
BF16 = mybir.dt.bfloat16
F8 = mybir.dt.float8e4
ALU = mybir.AluOpType
ACTF = mybir.ActivationFunctionType
DRM = mybir.MatmulPerfMode.DoubleRow

# Toggled by test.py for profiling runs.
PROFILE = False
TRACE_KWARGS = {}
LAST_RESULT = [None]

_built = [None]


def _split_waits(nc, cap=1):
    """Walrus in this container rejects >1 sync-wait per instruction.
    Move excess waits onto preceding same-engine NOPs (in-order engines,
    so semantics are preserved)."""
    n_new = 0
    for fn in nc.m.functions:
        for bb in fn.blocks:
            new_list = []
            for ins in bb.instructions:
                si = getattr(ins, "sync_info", None)
                if si is not None and si.on_wait and len(si.on_wait) > cap:
                    waits = list(si.on_wait)
                    keep, rest = waits[:cap], waits[cap:]
                    for i in range(0, len(rest), cap):
                        nop = mybir.InstNoOp(
                            name=f"I-waitsplit-{n_new}", ins=[], outs=[]
                        )
                        n_new += 1
                        nop.engine = ins.engine
                        nop.sync_info = mybir.SyncInfo(
                            on_wait=rest[i : i + cap], on_update=[]
                        )
                        new_list.append(nop)
                    si.on_wait = keep
                new_list.append(ins)
            bb.instructions = new_list
    return n_new


def _build():
    nc = bass.Bass()
    ea_ext = nc.declare_dram_parameter("ea", [PR, P, 2 * FA], F8, isOutput=False)
    eb_ext = nc.declare_dram_parameter("eb", [PR, P, 2 * D], BF16, isOutput=False)
    et_ext = nc.declare_dram_parameter("et", [P, 4 * NL], F8, isOutput=False)
    yf_ext = nc.declare_dram_parameter("yf", [P, KT], F32, isOutput=False)
    ctr_ext = nc.declare_dram_parameter("ctr", [CH, 1], F32, isOutput=False)
    p0s_ext = nc.declare_dram_parameter("p0s", [CH, D], F32, isOutput=False)
    out_ext = nc.declare_dram_parameter("out", [NL, C], F32, isOutput=True)

    with tile.TileContext(nc) as tc, ExitStack() as es:
        cpool = es.enter_context(tc.tile_pool(name="const", bufs=1))
        bpool = es.enter_context(tc.tile_pool(name="bigs", bufs=1))
        bigp = es.enter_context(tc.tile_pool(name="bigp", bufs=1))
        ebp = es.enter_context(tc.tile_pool(name="ebp", bufs=20))
        sqp = es.enter_context(tc.tile_pool(name="sqp", bufs=2))
        rp = es.enter_context(tc.tile_pool(name="rp", bufs=1))
        otp = es.enter_context(tc.tile_pool(name="otp", bufs=8))
        dram = es.enter_context(tc.tile_pool(name="dram", bufs=1, space="DRAM"))

        # ---- constants ----
        iota = cpool.tile([P, CP], F32, name="iota")
        nc.gpsimd.iota(
            iota[:], pattern=[[1, CP]], base=0, channel_multiplier=0,
            allow_small_or_imprecise_dtypes=True,
        )
        identb = cpool.tile([P, P], BF16, name="identb")
        make_identity(nc, identb[:])
        ones2b = cpool.tile([2, 2 * P], F8, name="ones2b")
        nc.vector.memset(ones2b[:], 0.0)
        nc.vector.memset(ones2b[0:1, 0:P], 1.0)

        y_sb = cpool.tile([P, KT], F32, name="y")
        nc.sync.dma_start(y_sb[:], yf_ext[:])
        ctr_sb = rp.tile([CH, 1], F32, name="ctr")
        nc.sync.dma_start(ctr_sb[:], ctr_ext[:])
        p0s_sb = cpool.tile([CH, D], F32, name="p0s")
        nc.sync.dma_start(p0s_sb[:], p0s_ext[:])
        e_sq = cpool.tile([P, KT], F32, name="esq")

        # ---- big resident buffers ----
        # ea_full (phase 1) and embT (phase 2) are never live at the same
        # time: share one pool slot (same tag) to free 32KB/partition.
        ea_full = bigp.tile([P, PR * 2 * FA], F8, tag="big", name="ea")
        eav = ea_full.rearrange("p (pr j f) -> p pr j f", pr=PR, j=2)
        oh_full = bpool.tile([P, KT * CP], F8, name="oh")
        ohv = oh_full.rearrange("p (pr j c) -> p pr j c", pr=PR, j=2)
        p2sb = bpool.tile([P, 4 * CP], F8, name="p2sb")
        p2v = p2sb.rearrange("p (dc c) -> p dc c", dc=4)  # c = CP
        psq2b = cpool.tile([2, 2 * CP], F8, name="psq2b")
        nc.vector.memset(psq2b[:], 0.0)
        ss = bpool.tile([P, 8 * FU], BF16, name="ss")
        ssv = ss.rearrange("p (cc f) -> p cc f", cc=8)

        # collective DRAM buffers
        ccin = dram.tile([CP, FU], BF16, name="ccin")
        rsout = dram.tile([CH, FU], BF16, name="rsout")
        agin = dram.tile([513, P], F8, name="agin")
        agout = dram.tile([W * 513, P], F8, name="agout", addr_space="Shared")

        # ---- phase 1: loads + one-hot ----
        eb_tiles = []
        for pr in range(PR):
            nc.sync.dma_start(
                eav[:, pr, :, :], ea_ext[pr]
            )
            for j in (0, 1):
                kt = 2 * pr + j
                dst = ohv[:, pr, j, :]
                # split one-hot generation DVE : Pool roughly 39:25
                eng = nc.vector if (kt % 16) < 10 else nc.gpsimd
                eng.tensor_scalar(dst, iota[:], y_sb[:, kt : kt + 1], None,
                                  ALU.is_equal)

        # counter-only coefficient work hoisted ahead of the ReduceScatter
        rt2 = rp.tile([CH, 1], F32, name="rt2")
        nc.vector.tensor_scalar(rt2[:], ctr_sb[:], 1.0, None, ALU.add)
        nc.vector.reciprocal(rt2[:], rt2[:])
        A2p = rp.tile([CH, 1], F32, name="A2p")
        nc.vector.tensor_tensor(out=A2p[:], in0=ctr_sb[:], in1=rt2[:], op=ALU.mult)
        nc.vector.tensor_scalar(A2p[:], A2p[:], 1.0, None, ALU.subtract)
        nc.vector.tensor_scalar(A2p[:], A2p[:], 2.0, None, ALU.mult)
        nc.vector.tensor_scalar(rt2[:], rt2[:], 2.0, None, ALU.mult)
        twos_c = rp.tile([CH, 1], F32, name="twosc")
        nc.vector.memset(twos_c[:], 2.0)


        # ---- phase 1: segment sums via DoubleRow fp8 ----
        # pr-major main pass over all 8 class chunks (8 PSUM banks),
        # pipelined with one-hot generation; the last 4 row-pairs of each
        # chunk run as staggered per-chunk tails so copies, counts matmuls
        # (bank reuse) and ccin DMAs pipeline instead of all colliding at
        # the end of the pass.
        PRM = PR - 6
        with tc.tile_pool(name="ps_seg", bufs=1, space="PSUM") as psg:
            psAs = {cc: psg.tile([CH, 512], F32, tag=f"psA{cc}",
                                 name=f"psA{cc}") for cc in range(8)}
            for pr in range(PRM):
                for cc in range(8):
                    nc.tensor.matmul(
                        psAs[cc][:], ohv[:, pr, :, cc * CH : (cc + 1) * CH],
                        eav[:, pr, :, 0:512],
                        start=(pr == 0), stop=False, perf_mode=DRM,
                    )
            for cc in range(8):
                for pr in range(PRM, PR):
                    nc.tensor.matmul(
                        psAs[cc][:], ohv[:, pr, :, cc * CH : (cc + 1) * CH],
                        eav[:, pr, :, 0:512],
                        start=False, stop=(pr == PR - 1), perf_mode=DRM,
                    )
                # sums copy on DVE (Act runs the e_sq squares; Pool must stay
                # clear so the ReduceScatter can start early)
                nc.vector.tensor_copy(out=ssv[:, cc, 0:512], in_=psAs[cc][:])
                # counts: reuse chunk cc's bank (same tag) once copied out
                psB = psg.tile([CH, 512], F32, tag=f"psA{cc}", name=f"psB{cc}")
                for pr in range(PR):
                    nc.tensor.matmul(
                        psB[:, 0:4], ohv[:, pr, :, cc * CH : (cc + 1) * CH],
                        eav[:, pr, :, 512:516],
                        start=(pr == 0), stop=(pr == PR - 1), perf_mode=DRM,
                    )
                nc.vector.tensor_copy(out=ssv[:, cc, 512:516], in_=psB[:, 0:4])
                # ccin DMAs ride the Pool queue (SP is busy with eb loads;
                # Pool is idle between one-hot gen and the ReduceScatter)
                nc.gpsimd.dma_start(ccin[cc * CH : (cc + 1) * CH, :],
                                    ssv[:, cc, :])

        # ---- ReduceScatter (sums+counts, bf16) ----
        nc.gpsimd.collective_compute(
            "ReduceScatter", ALU.add,
            replica_groups=[list(range(W))],
            ins=[ccin.opt()], outs=[rsout.opt()],
        )

        # ---- e_sq: eb loads on SP; Square+accumulate on Act, which is
        # otherwise idle and keeps running through the collective windows ----
        for pr in range(PR):
            ebt = ebp.tile([P, 2 * D], BF16, tag="eb", name="eb")
            nc.sync.dma_start(ebt[:], eb_ext[pr])
            eb_tiles.append(ebt)
            for j in (0, 1):
                kt = 2 * pr + j
                scr = sqp.tile([P, D], BF16, tag="scr", name="scr")
                nc.scalar.activation(
                    scr[:], ebt[:, j * D : (j + 1) * D], ACTF.Square,
                    accum_out=e_sq[:, kt : kt + 1],
                )

        # negate e_sq once (used as per-partition bias in phase 2)
        nc.scalar.mul(e_sq[:], e_sq[:], -1.0)

        # ---- embT load into ea_full's slot (overlaps the ReduceScatter) ----
        embT = bigp.tile([P, PR * 2 * FA], F8, tag="big", name="embT")
        etv = embT.rearrange("p (q n) -> p q n", q=4)[:, :, 0:NL]
        for q in range(4):
            nc.sync.dma_start(etv[:, q, :], et_ext[:, q * NL : (q + 1) * NL])

        # ---- per-rank prototype update (128 classes incl. padding) ----
        # B2 = 2*rep*rm*rt ; A2 = 2*(1 + rep*(ctr*rt - 1)); rt2=2rt and
        # A2p=ctr*rt-1 were precomputed before the ReduceScatter.
        shard = rp.tile([CH, FU], BF16, name="shard")
        nc.sync.dma_start(shard[:], rsout[:])
        counts = shard[:, 512:513]
        rm = rp.tile([CH, 1], F32, name="rm")
        nc.vector.tensor_scalar(rm[:], counts, 1.0, None, ALU.max)
        nc.vector.reciprocal(rm[:], rm[:])
        rep = rp.tile([CH, 1], F32, name="rep")
        nc.vector.tensor_scalar(rep[:], counts, 0.0, None, ALU.is_gt)
        B2 = rp.tile([CH, 1], F32, name="B2")
        nc.vector.scalar_tensor_tensor(
            out=B2[:], in0=rm[:], scalar=rt2[:], in1=rep[:],
            op0=ALU.mult, op1=ALU.mult,
        )
        A2 = rp.tile([CH, 1], F32, name="A2")
        nc.vector.scalar_tensor_tensor(
            out=A2[:], in0=A2p[:], scalar=rep[:], in1=twos_c[:],
            op0=ALU.mult, op1=ALU.add,
        )

        tB = rp.tile([CH, D], F32, name="tB")
        nc.vector.tensor_scalar(tB[:], shard[:, 0:512], B2[:], None, ALU.mult)
        p2t_b = rp.tile([CH, D], BF16, name="p2tb")
        nc.vector.scalar_tensor_tensor(
            out=p2t_b[:], in0=p0s_sb[:], scalar=A2[:], in1=tB[:],
            op0=ALU.mult, op1=ALU.add,
        )
        # -p_sq = -0.25 * sum_d protos2^2
        scr2 = rp.tile([CH, D], BF16, name="scr2")
        npsq_b = rp.tile([CH, 1], BF16, name="npsqb")
        nc.vector.scalar_tensor_tensor(
            out=scr2[:], in0=p2t_b[:], scalar=-0.25, in1=p2t_b[:],
            op0=ALU.mult, op1=ALU.mult, accum_out=npsq_b[:],
        )

        # transpose this rank's protos2T to [d, c] BEFORE the AllGather so
        # no transpose work sits on the post-collective critical path
        agst = rp.tile([P, 4 * P], F8, name="agst")
        agsv = agst.rearrange("p (dc c) -> p dc c", dc=4)
        psq_st = rp.tile([1, P], F8, name="psqst")
        with tc.tile_pool(name="ps_tr", bufs=1, space="PSUM") as pst:
            t2 = pst.tile([P, 4 * P], BF16, tag="t2", name="t2")
            t2v = t2.rearrange("p (dc c) -> p dc c", dc=4)
            for dc in range(4):
                nc.tensor.matmul(
                    t2v[:, dc, :], p2t_b[:, dc * P : (dc + 1) * P], identb[:],
                    is_transpose=True, start=(dc == 0), stop=(dc == 3),
                )
            tq2 = pst.tile([1, P], BF16, tag="tq2", name="tq2")
            nc.tensor.matmul(tq2[:], npsq_b[:], identb[:],
                             is_transpose=True, start=True, stop=True)
            nc.vector.tensor_copy(out=agst[:], in_=t2[:])
            nc.vector.tensor_copy(out=psq_st[:], in_=tq2[:])
        # agin rows 0..511 = protos2 chunk [d, c]; row 512 = -p_sq row
        # (on Pool: SP is still draining the eb/embT streams at this point)
        nc.gpsimd.dma_start(
            agin[0:512, :].rearrange("(dc p) c -> p dc c", dc=4),
            agsv[:, :, :],
        )
        nc.gpsimd.dma_start(agin[512:513, :], psq_st[:])

        # ---- AllGather (protos2T + -p_sq, fp8) ----
        nc.gpsimd.collective_compute(
            "AllGather", ALU.bypass,
            replica_groups=[list(range(W))],
            ins=[agin.opt()], outs=[agout.opt()],
        )

        # ---- load gathered protos2 (already [d, c] per rank) + -p_sq row ----
        # psq row first: the p_sq seed matmul opens every accumulation group
        agov = agout.rearrange("(r q) c -> q r c", r=W)
        nc.sync.dma_start(
            psq2b[0:1, 0:CP].rearrange("a (r c) -> a r c", r=W),
            agov[512:513, :, :],
        )
        for dc in range(4):
            eng = nc.scalar if dc < 2 else nc.sync
            eng.dma_start(
                p2sb.rearrange("p (dc r c) -> p dc r c", dc=4, r=W)[:, dc, :, :],
                agov[dc * P : (dc + 1) * P, :, :],
            )

        # ---- phase 2: out = 2*emb@protosT - e_sq - p_sq ----
        with tc.tile_pool(name="ps_cr", bufs=4, space="PSUM") as ps_cr:
            for nt in range(KT):
                ot = otp.tile([P, CP], F32, tag="ot", name="ot")
                for h in range(2):
                    c0 = 512 * h
                    cr = ps_cr.tile([P, 512], F32, tag=f"cr{h}", name=f"cr{h}")
                    nc.tensor.matmul(
                        cr[:],
                        ones2b.rearrange("k (j m) -> k j m", j=2)[:, :, :],
                        psq2b.rearrange("k (j c) -> k j c", j=2)[:, :, c0 : c0 + 512],
                        start=True, stop=False, perf_mode=DRM,
                    )
                    for q in range(2):
                        nc.tensor.matmul(
                            cr[:],
                            etv[:, 2 * q : 2 * q + 2, nt * P : (nt + 1) * P],
                            p2v[:, 2 * q : 2 * q + 2, c0 : c0 + 512],
                            start=False, stop=(q == 1), perf_mode=DRM,
                        )
                    # epilogue: add -e_sq while copying psum -> sbuf
                    # (gpsimd cannot access PSUM, so Act/DVE only)
                    if (2 * nt + h) % 9 in (0, 2, 4, 6):
                        nc.scalar.activation(
                            ot[:, c0 : c0 + 512], cr[:], ACTF.Identity,
                            bias=e_sq[:, nt : nt + 1], scale=1.0,
                        )
                    else:
                        nc.vector.tensor_scalar(
                            ot[:, c0 : c0 + 512], cr[:], e_sq[:, nt : nt + 1],
                            None, ALU.add,
                        )
                # output DMA rotation, finely interleaved: SP ~28, Pool ~28, Act ~8
                r = nt % 9
                if r in (0, 2, 4, 6):
                    eng = nc.sync
                elif r in (1, 3, 5, 7):
                    eng = nc.gpsimd
                else:
                    eng = nc.scalar
                eng.dma_start(out_ext[nt * P : (nt + 1) * P, :], ot[:, 0:C])

    _split_waits(nc)
    return nc


def _prep_inputs(embeddings, prototypes, counter, y_true):
    """Host-side sharding + layout prep (no kernel math beyond dtype casts)."""
    emb = np.ascontiguousarray(np.asarray(embeddings, dtype=np.float32))
    p0 = np.ascontiguousarray(np.asarray(prototypes, dtype=np.float32))
    ctr = np.ascontiguousarray(np.asarray(counter, dtype=np.float32))
    y = np.asarray(y_true)

    f8 = ml_dtypes.float8_e4m3
    bf = ml_dtypes.bfloat16

    p0_pad = np.zeros((CP, D), dtype=np.float32)
    p0_pad[0:C] = p0
    ctr_pad = np.zeros((CP,), dtype=np.float32)
    ctr_pad[0:C] = ctr

    in_maps = []
    for i in range(W):
        sl = slice(i * NL, (i + 1) * NL)
        e_i = emb[sl]                                   # [NL, D] f32
        # emb_aug fp8 pairs, partition-major: [PR, P, 2*FA]
        ea = np.zeros((NL, FA), dtype=f8)
        ea[:, 0:D] = e_i.astype(f8)
        ea[:, D] = 1.0
        ea_t = np.ascontiguousarray(
            ea.reshape(PR, 2, P, FA).transpose(0, 2, 1, 3).reshape(PR, P, 2 * FA)
        )
        # bf16 pairs for e_sq: [PR, P, 2*D]
        eb = e_i.astype(bf)
        eb_t = np.ascontiguousarray(
            eb.reshape(PR, 2, P, D).transpose(0, 2, 1, 3).reshape(PR, P, 2 * D)
        )
        # embT fp8: [P, 4*NL] with et[k, dc*NL + n] = emb[n, 128*dc + k]
        et = np.ascontiguousarray(
            e_i.astype(f8).T.reshape(4, P, NL).transpose(1, 0, 2).reshape(P, 4 * NL)
        )
        # labels, partition-major: yf[p, t] = y[t*128 + p]
        y_loc = y[sl].astype(np.float32)
        yf = np.ascontiguousarray(y_loc.reshape(KT, P).T)
        # per-rank class shard (class axis padded to CP)
        cs = slice(i * CH, (i + 1) * CH)
        in_maps.append(
            {
                "ea": ea_t,
                "eb": eb_t,
                "et": et,
                "yf": yf,
                "ctr": np.ascontiguousarray(ctr_pad[cs]).reshape(CH, 1),
                "p0s": np.ascontiguousarray(p0_pad[cs]),
            }
        )
    return in_maps


def kernel(embeddings, prototypes, counter, y_true):
    if _built[0] is None:
        _built[0] = _build()
    nc = _built[0]

    in_maps = _prep_inputs(embeddings, prototypes, counter, y_true)

    res = run_bass_kernel_spmd(
        nc, in_maps, list(range(W)), trace=PROFILE, **TRACE_KWARGS
    )
    LAST_RESULT[0] = res
    out = np.concatenate([res.results[i]["out"] for i in range(W)], axis=0)
    return out.astype(np.float32, copy=False)


# revision 82
# speedup vs baseline: 1.0208x; 1.0090x over previous
"""DeepNCM Trainium2 kernel: prototype scatter-mean update + negative squared
L2 distances, data-parallel over embedding rows across 8 NeuronCores.

Contract: kernel(**inputs) takes the FULL unsharded inputs
(embeddings [65536,512] f32, prototypes [1000,512] f32, counter [1000] f32,
y_true [65536] int64) and returns the FULL output [65536,1000] f32.

Per-core plan (NL = 8192 rows, class axis padded to 1024 = 8 chunks of 128,
fp8 DoubleRow matmuls throughout; ~146us in the CoreSim cost model vs 365us
for the bf16 AllReduce baseline):
  Phase 1: sumsT[c, d] = oh^T @ emb_aug via DoubleRow fp8 matmuls (two
     row-tiles = 256 contraction rows per instruction), one pr-major pass
     over all 8 class chunks (8 PSUM banks), pipelined with one-hot
     generation (DVE+Pool). emb_aug carries a ones column; the per-class
     counts matmuls reuse each chunk's bank as its sums copy drains it.
     e_sq comes from bf16 emb via Act Square+accumulate, running through
     the collective windows.
  ReduceScatter [1024,516] bf16 -> each rank owns 128 classes (sums+counts).
  Per-rank prototype update + PE-transpose to [d, c] BEFORE the AllGather,
  so the fp8 AG payload [513,128] lands in matmul-ready layout (plus a
  -p_sq row); nothing but strided loads remains after the AG.
  Phase 2: out = 2*emb@protosT - e_sq - p_sq: each PSUM half accumulates a
     K=2 DoubleRow matmul seeding -p_sq, then 2 DoubleRow fp8 matmuls
     (embT pairs x protos2); the epilogue copy adds -e_sq (per-partition
     bias/scalar) split Act/DVE; output DMAs rotate across SP/Pool/Act.
"""

import os
import sys
from contextlib import ExitStack

for _p in ("/opt/trn_rl_repo", "/root/.axon_site/_ro/trn_rl_repo"):
    if os.path.isdir(_p):
        if _p not in sys.path:
            sys.path.insert(0, _p)
        break

import numpy as np
import ml_dtypes

import concourse.bass as bass
import concourse.mybir as mybir
import concourse.tile as tile
from concourse.masks import make_identity
from concourse.bass_utils import run_bass_kernel_spmd

N, D, C = 65536, 512, 1000
W = 8                      # cores
NL = N // W                # 8192 rows per core
P = 128
KT = NL // P               # 64 row tiles per core
PR = KT // 2               # 32 row-tile pairs (DoubleRow)
CP = 1024                  # class axis padded to 8 chunks of 128
CH = 128                   # classes per rank / class chunk (incl. padding)
FA = 520                   # emb_aug width: 512 emb + ones col + 7 pad
FU = 516                   # used width in collective buffers
F32 = mybir.dt.float32
BF16 = mybir.dt.bfloat16
F8 = mybir.dt.float8e4
ALU = mybir.AluOpType
ACTF = mybir.ActivationFunctionType
DRM = mybir.MatmulPerfMode.DoubleRow

# Toggled by test.py for profiling runs.
PROFILE = False
TRACE_KWARGS = {}
LAST_RESULT = [None]

_built = [None]


def _split_waits(nc, cap=1):
    """Walrus in this container rejects >1 sync-wait per instruction.
    Move excess waits onto preceding same-engine NOPs (in-order engines,
    so semantics are preserved)."""
    n_new = 0
    for fn in nc.m.functions:
        for bb in fn.blocks:
            new_list = []
            for ins in bb.instructions:
                si = getattr(ins, "sync_info", None)
                if si is not None and si.on_wait and len(si.on_wait) > cap:
                    waits = list(si.on_wait)
                    keep, rest = waits[:cap], waits[cap:]
                    for i in range(0, len(rest), cap):
                        nop = mybir.InstNoOp(
                            name=f"I-waitsplit-{n_new}", ins=[], outs=[]
                        )
                        n_new += 1
                        nop.engine = ins.engine
                        nop.sync_info = mybir.SyncInfo(
                            on_wait=rest[i : i + cap], on_update=[]
                        )
                        new_list.append(nop)
                    si.on_wait = keep
                new_list.append(ins)
            bb.instructions = new_list
    return n_new


def _build():
    nc = bass.Bass()
    ea_ext = nc.declare_dram_parameter("ea", [PR, P, 2 * FA], F8, isOutput=False)
    eb_ext = nc.declare_dram_parameter("eb", [PR, P, 2 * D], BF16, isOutput=False)
    et_ext = nc.declare_dram_parameter("et", [P, 4 * NL], F8, isOutput=False)
    yf_ext = nc.declare_dram_parameter("yf", [P, KT], F32, isOutput=False)
    ctr_ext = nc.declare_dram_parameter("ctr", [CH, 1], F32, isOutput=False)
    p0s_ext = nc.declare_dram_parameter("p0s", [CH, D], F32, isOutput=False)
    out_ext = nc.declare_dram_parameter("out", [NL, C], F32, isOutput=True)

    with tile.TileContext(nc) as tc, ExitStack() as es:
        cpool = es.enter_context(tc.tile_pool(name="const", bufs=1))
        bpool = es.enter_context(tc.tile_pool(name="bigs", bufs=1))
        bigp = es.enter_context(tc.tile_pool(name="bigp", bufs=1))
        ebp = es.enter_context(tc.tile_pool(name="ebp", bufs=20))
        sqp = es.enter_context(tc.tile_pool(name="sqp", bufs=2))
        rp = es.enter_context(tc.tile_pool(name="rp", bufs=1))
        otp = es.enter_context(tc.tile_pool(name="otp", bufs=8))
        dram = es.enter_context(tc.tile_pool(name="dram", bufs=1, space="DRAM"))

        # ---- constants ----
        iota = cpool.tile([P, CP], F32, name="iota")
        nc.gpsimd.iota(
            iota[:], pattern=[[1, CP]], base=0, channel_multiplier=0,
            allow_small_or_imprecise_dtypes=True,
        )
        identb = cpool.tile([P, P], BF16, name="identb")
        make_identity(nc, identb[:])
        ones2b = cpool.tile([2, 2 * P], F8, name="ones2b")
        nc.vector.memset(ones2b[:], 0.0)
        nc.vector.memset(ones2b[0:1, 0:P], 1.0)

        y_sb = cpool.tile([P, KT], F32, name="y")
        nc.sync.dma_start(y_sb[:], yf_ext[:])
        ctr_sb = rp.tile([CH, 1], F32, name="ctr")
        nc.sync.dma_start(ctr_sb[:], ctr_ext[:])
        p0s_sb = cpool.tile([CH, D], F32, name="p0s")
        nc.sync.dma_start(p0s_sb[:], p0s_ext[:])
        e_sq = cpool.tile([P, KT], F32, name="esq")

        # ---- big resident buffers ----
        # ea_full (phase 1) and embT (phase 2) are never live at the same
        # time: share one pool slot (same tag) to free 32KB/partition.
        ea_full = bigp.tile([P, PR * 2 * FA], F8, tag="big", name="ea")
        eav = ea_full.rearrange("p (pr j f) -> p pr j f", pr=PR, j=2)
        oh_full = bpool.tile([P, KT * CP], F8, name="oh")
        ohv = oh_full.rearrange("p (pr j c) -> p pr j c", pr=PR, j=2)
        p2sb = bpool.tile([P, 4 * CP], F8, name="p2sb")
        p2v = p2sb.rearrange("p (dc c) -> p dc c", dc=4)  # c = CP
        psq2b = cpool.tile([2, 2 * CP], F8, name="psq2b")
        nc.vector.memset(psq2b[:], 0.0)
        ss = bpool.tile([P, 8 * FU], BF16, name="ss")
        ssv = ss.rearrange("p (cc f) -> p cc f", cc=8)

        # collective DRAM buffers
        ccin = dram.tile([CP, FU], BF16, name="ccin")
        rsout = dram.tile([CH, FU], BF16, name="rsout")
        agin = dram.tile([513, P], F8, name="agin")
        agout = dram.tile([W * 513, P], F8, name="agout", addr_space="Shared")

        # ---- phase 1: loads + one-hot ----
        eb_tiles = []
        for pr in range(PR):
            nc.sync.dma_start(
                eav[:, pr, :, :], ea_ext[pr]
            )
            for j in (0, 1):
                kt = 2 * pr + j
                dst = ohv[:, pr, j, :]
                # split one-hot generation DVE : Pool roughly 39:25
                eng = nc.vector if (kt % 16) < 10 else nc.gpsimd
                eng.tensor_scalar(dst, iota[:], y_sb[:, kt : kt + 1], None,
                                  ALU.is_equal)

        # counter-only coefficient work hoisted ahead of the ReduceScatter
        rt2 = rp.tile([CH, 1], F32, name="rt2")
        nc.vector.tensor_scalar(rt2[:], ctr_sb[:], 1.0, None, ALU.add)
        nc.vector.reciprocal(rt2[:], rt2[:])
        A2p = rp.tile([CH, 1], F32, name="A2p")
        nc.vector.tensor_tensor(out=A2p[:], in0=ctr_sb[:], in1=rt2[:], op=ALU.mult)
        nc.vector.tensor_scalar(A2p[:], A2p[:], 1.0, None, ALU.subtract)
        nc.vector.tensor_scalar(A2p[:], A2p[:], 2.0, None, ALU.mult)
        nc.vector.tensor_scalar(rt2[:], rt2[:], 2.0, None, ALU.mult)
        twos_c = rp.tile([CH, 1], F32, name="twosc")
        nc.vector.memset(twos_c[:], 2.0)


        # ---- phase 1: segment sums via DoubleRow fp8 ----
        # pr-major main pass over all 8 class chunks (8 PSUM banks),
        # pipelined with one-hot generation; the last 4 row-pairs of each
        # chunk run as staggered per-chunk tails so copies, counts matmuls
        # (bank reuse) and ccin DMAs pipeline instead of all colliding at
        # the end of the pass.
        PRM = PR - 6
        with tc.tile_pool(name="ps_seg", bufs=1, space="PSUM") as psg:
            psAs = {cc: psg.tile([CH, 512], F32, tag=f"psA{cc}",
                                 name=f"psA{cc}") for cc in range(8)}
            for pr in range(PRM):
                for cc in range(8):
                    nc.tensor.matmul(
                        psAs[cc][:], ohv[:, pr, :, cc * CH : (cc + 1) * CH],
                        eav[:, pr, :, 0:512],
                        start=(pr == 0), stop=False, perf_mode=DRM,
                    )
            for cc in range(8):
                for pr in range(PRM, PR):
                    nc.tensor.matmul(
                        psAs[cc][:], ohv[:, pr, :, cc * CH : (cc + 1) * CH],
                        eav[:, pr, :, 0:512],
                        start=False, stop=(pr == PR - 1), perf_mode=DRM,
                    )
                # sums copy on DVE (Act runs the e_sq squares; Pool must stay
                # clear so the ReduceScatter can start early)
                nc.vector.tensor_copy(out=ssv[:, cc, 0:512], in_=psAs[cc][:])
                # counts: reuse chunk cc's bank (same tag) once copied out
                psB = psg.tile([CH, 512], F32, tag=f"psA{cc}", name=f"psB{cc}")
                for pr in range(PR):
                    nc.tensor.matmul(
                        psB[:, 0:4], ohv[:, pr, :, cc * CH : (cc + 1) * CH],
                        eav[:, pr, :, 512:516],
                        start=(pr == 0), stop=(pr == PR - 1), perf_mode=DRM,
                    )
                nc.vector.tensor_copy(out=ssv[:, cc, 512:516], in_=psB[:, 0:4])
                # ccin DMAs ride the Pool queue (SP is busy with eb loads;
                # Pool is idle between one-hot gen and the ReduceScatter)
                nc.gpsimd.dma_start(ccin[cc * CH : (cc + 1) * CH, :],
                                    ssv[:, cc, :])

        # ---- ReduceScatter (sums+counts, bf16) ----
        nc.gpsimd.collective_compute(
            "ReduceScatter", ALU.add,
            replica_groups=[list(range(W))],
            ins=[ccin.opt()], outs=[rsout.opt()],
        )

        # ---- e_sq: eb loads on SP; Square+accumulate on Act, which is
        # otherwise idle and keeps running through the collective windows ----
        for pr in range(PR):
            ebt = ebp.tile([P, 2 * D], BF16, tag="eb", name="eb")
            nc.sync.dma_start(ebt[:], eb_ext[pr])
            eb_tiles.append(ebt)
            for j in (0, 1):
                kt = 2 * pr + j
                scr = sqp.tile([P, D], BF16, tag="scr", name="scr")
                nc.scalar.activation(
                    scr[:], ebt[:, j * D : (j + 1) * D], ACTF.Square,
                    accum_out=e_sq[:, kt : kt + 1],
                )

        # negate e_sq once (used as per-partition bias in phase 2)
        nc.scalar.mul(e_sq[:], e_sq[:], -1.0)

        # ---- embT load into ea_full's slot (overlaps the ReduceScatter) ----
        embT = bigp.tile([P, PR * 2 * FA], F8, tag="big", name="embT")
        etv = embT.rearrange("p (q n) -> p q n", q=4)[:, :, 0:NL]
        for q in range(4):
            nc.sync.dma_start(etv[:, q, :], et_ext[:, q * NL : (q + 1) * NL])

        # ---- per-rank prototype update (128 classes incl. padding) ----
        # B2 = 2*rep*rm*rt ; A2 = 2*(1 + rep*(ctr*rt - 1)); rt2=2rt and
        # A2p=ctr*rt-1 were precomputed before the ReduceScatter.
        shard = rp.tile([CH, FU], BF16, name="shard")
        nc.gpsimd.dma_start(shard[:], rsout[:])
        counts = shard[:, 512:513]
        rm = rp.tile([CH, 1], F32, name="rm")
        nc.vector.tensor_scalar(rm[:], counts, 1.0, None, ALU.max)
        nc.vector.reciprocal(rm[:], rm[:])
        rep = rp.tile([CH, 1], F32, name="rep")
        nc.vector.tensor_scalar(rep[:], counts, 0.0, None, ALU.is_gt)
        B2 = rp.tile([CH, 1], F32, name="B2")
        nc.vector.scalar_tensor_tensor(
            out=B2[:], in0=rm[:], scalar=rt2[:], in1=rep[:],
            op0=ALU.mult, op1=ALU.mult,
        )
        A2 = rp.tile([CH, 1], F32, name="A2")
        nc.vector.scalar_tensor_tensor(
            out=A2[:], in0=A2p[:], scalar=rep[:], in1=twos_c[:],
            op0=ALU.mult, op1=ALU.add,
        )

        tB = rp.tile([CH, D], F32, name="tB")
        nc.vector.tensor_scalar(tB[:], shard[:, 0:512], B2[:], None, ALU.mult)
        p2t_b = rp.tile([CH, D], BF16, name="p2tb")
        nc.vector.scalar_tensor_tensor(
            out=p2t_b[:], in0=p0s_sb[:], scalar=A2[:], in1=tB[:],
            op0=ALU.mult, op1=ALU.add,
        )
        # -p_sq = -0.25 * sum_d protos2^2
        scr2 = rp.tile([CH, D], BF16, name="scr2")
        npsq_b = rp.tile([CH, 1], BF16, name="npsqb")
        nc.vector.scalar_tensor_tensor(
            out=scr2[:], in0=p2t_b[:], scalar=-0.25, in1=p2t_b[:],
            op0=ALU.mult, op1=ALU.mult, accum_out=npsq_b[:],
        )

        # transpose this rank's protos2T to [d, c] BEFORE the AllGather so
        # no transpose work sits on the post-collective critical path
        agst = rp.tile([P, 4 * P], F8, name="agst")
        agsv = agst.rearrange("p (dc c) -> p dc c", dc=4)
        psq_st = rp.tile([1, P], F8, name="psqst")
        with tc.tile_pool(name="ps_tr", bufs=1, space="PSUM") as pst:
            t2 = pst.tile([P, 4 * P], BF16, tag="t2", name="t2")
            t2v = t2.rearrange("p (dc c) -> p dc c", dc=4)
            for dc in range(4):
                nc.tensor.matmul(
                    t2v[:, dc, :], p2t_b[:, dc * P : (dc + 1) * P], identb[:],
                    is_transpose=True, start=(dc == 0), stop=(dc == 3),
                )
            tq2 = pst.tile([1, P], BF16, tag="tq2", name="tq2")
            nc.tensor.matmul(tq2[:], npsq_b[:], identb[:],
                             is_transpose=True, start=True, stop=True)
            nc.vector.tensor_copy(out=agst[:], in_=t2[:])
            nc.vector.tensor_copy(out=psq_st[:], in_=tq2[:])
        # agin rows 0..511 = protos2 chunk [d, c]; row 512 = -p_sq row
        # (on Pool: SP is still draining the eb/embT streams at this point)
        nc.gpsimd.dma_start(
            agin[0:512, :].rearrange("(dc p) c -> p dc c", dc=4),
            agsv[:, :, :],
        )
        nc.gpsimd.dma_start(agin[512:513, :], psq_st[:])

        # ---- AllGather (protos2T + -p_sq, fp8) ----
        nc.gpsimd.collective_compute(
            "AllGather", ALU.bypass,
            replica_groups=[list(range(W))],
            ins=[agin.opt()], outs=[agout.opt()],
        )

        # ---- load gathered protos2 (already [d, c] per rank) + -p_sq row ----
        # psq row first: the p_sq seed matmul opens every accumulation group
        agov = agout.rearrange("(r q) c -> q r c", r=W)
        nc.sync.dma_start(
            psq2b[0:1, 0:CP].rearrange("a (r c) -> a r c", r=W),
            agov[512:513, :, :],
        )
        for dc in range(4):
            eng = nc.scalar if dc < 2 else nc.sync
            eng.dma_start(
                p2sb.rearrange("p (dc r c) -> p dc r c", dc=4, r=W)[:, dc, :, :],
                agov[dc * P : (dc + 1) * P, :, :],
            )

        # ---- phase 2: out = 2*emb@protosT - e_sq - p_sq ----
        with tc.tile_pool(name="ps_cr", bufs=4, space="PSUM") as ps_cr:
            for nt in range(KT):
                ot = otp.tile([P, CP], F32, tag="ot", name="ot")
                for h in range(2):
                    c0 = 512 * h
                    cr = ps_cr.tile([P, 512], F32, tag=f"cr{h}", name=f"cr{h}")
                    nc.tensor.matmul(
                        cr[:],
                        ones2b.rearrange("k (j m) -> k j m", j=2)[:, :, :],
                        psq2b.rearrange("k (j c) -> k j c", j=2)[:, :, c0 : c0 + 512],
                        start=True, stop=False, perf_mode=DRM,
                    )
                    for q in range(2):
                        nc.tensor.matmul(
                            cr[:],
                            etv[:, 2 * q : 2 * q + 2, nt * P : (nt + 1) * P],
                            p2v[:, 2 * q : 2 * q + 2, c0 : c0 + 512],
                            start=False, stop=(q == 1), perf_mode=DRM,
                        )
                    # epilogue: add -e_sq while copying psum -> sbuf
                    # (gpsimd cannot access PSUM, so Act/DVE only)
                    if (2 * nt + h) % 9 in (0, 2, 4, 6):
                        nc.scalar.activation(
                            ot[:, c0 : c0 + 512], cr[:], ACTF.Identity,
                            bias=e_sq[:, nt : nt + 1], scale=1.0,
                        )
                    else:
                        nc.vector.tensor_scalar(
                            ot[:, c0 : c0 + 512], cr[:], e_sq[:, nt : nt + 1],
                            None, ALU.add,
                        )
                # output DMA rotation, finely interleaved: SP ~28, Pool ~28, Act ~8
                r = nt % 9
                if r in (0, 2, 4, 6):
                    eng = nc.sync
                elif r in (1, 3, 5, 7):
                    eng = nc.gpsimd
                else:
                    eng = nc.scalar
                eng.dma_start(out_ext[nt * P : (nt + 1) * P, :], ot[:, 0:C])

    _split_waits(nc)
    return nc


def _prep_inputs(embeddings, prototypes, counter, y_true):
    """Host-side sharding + layout prep (no kernel math beyond dtype casts)."""
    emb = np.ascontiguousarray(np.asarray(embeddings, dtype=np.float32))
    p0 = np.ascontiguousarray(np.asarray(prototypes, dtype=np.float32))
    ctr = np.ascontiguousarray(np.asarray(counter, dtype=np.float32))
    y = np.asarray(y_true)

    f8 = ml_dtypes.float8_e4m3
    bf = ml_dtypes.bfloat16

    p0_pad = np.zeros((CP, D), dtype=np.float32)
    p0_pad[0:C] = p0
    ctr_pad = np.zeros((CP,), dtype=np.float32)
    ctr_pad[0:C] = ctr

    in_maps = []
    for i in range(W):
        sl = slice(i * NL, (i + 1) * NL)
        e_i = emb[sl]                                   # [NL, D] f32
        # emb_aug fp8 pairs, partition-major: [PR, P, 2*FA]
        ea = np.zeros((NL, FA), dtype=f8)
        ea[:, 0:D] = e_i.astype(f8)
        ea[:, D] = 1.0
        ea_t = np.ascontiguousarray(
            ea.reshape(PR, 2, P, FA).transpose(0, 2, 1, 3).reshape(PR, P, 2 * FA)
        )
        # bf16 pairs for e_sq: [PR, P, 2*D]
        eb = e_i.astype(bf)
        eb_t = np.ascontiguousarray(
            eb.reshape(PR, 2, P, D).transpose(0, 2, 1, 3).reshape(PR, P, 2 * D)
        )
        # embT fp8: [P, 4*NL] with et[k, dc*NL + n] = emb[n, 128*dc + k]
        et = np.ascontiguousarray(
            e_i.astype(f8).T.reshape(4, P, NL).transpose(1, 0, 2).reshape(P, 4 * NL)
        )
        # labels, partition-major: yf[p, t] = y[t*128 + p]
        y_loc = y[sl].astype(np.float32)
        yf = np.ascontiguousarray(y_loc.reshape(KT, P).T)
        # per-rank class shard (class axis padded to CP)
        cs = slice(i * CH, (i + 1) * CH)
        in_maps.append(
            {
                "ea": ea_t,
                "eb": eb_t,
                "et": et,
                "yf": yf,
                "ctr": np.ascontiguousarray(ctr_pad[cs]).reshape(CH, 1),
                "p0s": np.ascontiguousarray(p0_pad[cs]),
            }
        )
    return in_maps


def kernel(embeddings, prototypes, counter, y_true):
    if _built[0] is None:
        _built[0] = _build()
    nc = _built[0]

    in_maps = _prep_inputs(embeddings, prototypes, counter, y_true)

    res = run_bass_kernel_spmd(
        nc, in_maps, list(range(W)), trace=PROFILE, **TRACE_KWARGS
    )
    LAST_RESULT[0] = res
    out = np.concatenate([res.results[i]["out"] for i in range(W)], axis=0)
    return out.astype(np.float32, copy=False)
